# revision 9
# baseline (speedup 1.0000x reference)
"""EMA recurrence kernel for Trainium2 (8 NeuronCores, batch-parallel).

Computes c[b,t,d] = x[b,t,d] + decay * c[b,t-1,d]  (decay = sigmoid(decay_logit))
for x of shape (8, 4096, 2048) fp32, as a blocked scan:

  - T is split into chunks of L=127 rows. Within a chunk the scan is a
    triangular matmul: out[t,d] = sum_{s<=t} decay^(t-s) x[s,d].
  - The cross-chunk carry (c at the last row of the previous chunk) is folded
    into the same matmul as an extra contraction row whose weight column is
    decay^(t+1) — so each chunk is ONE matmul per 512-wide D tile.  Matmuls
    run in float32r (single-pass fp32 PE mode, ~1e-4 rel err) instead of the
    2-pass exact fp32 mode, which would make PE the bottleneck.
  - Layout: the carry input row lives at SBUF partition 0 (x rows at
    partitions 1..127), and the matmul's output columns are permuted so that
    PSUM partition 0 holds the chunk's LAST scan position (the next carry)
    and partitions 1..127 hold scan positions 0..126.  All compute-engine
    access patterns therefore start at partition 0 (the BIR verifier rejects
    engine APs starting at non-32-aligned partitions); only DMA (which has
    no partition-alignment restriction) touches rows 1..127.  Chunk 0 has no
    carry: it uses its own weight matrix with x rows at partitions 0..126.
  - Carry copies run on ScalarE straight from PSUM (so the PE chain does not
    wait on VectorE's output copies); output copies run on VectorE.
  - DMA: chunks are grouped 4-per-dma_start (~4 MB contiguous-per-row 3D APs)
    on the SWDGE/gpsimd path — the only path that sprays descriptors across
    all 16 SDMA engines (HWDGE serializes everything onto one engine).
  - Batch b is sharded across the 8 cores (one b per core).
"""

import os
import sys

os.environ.setdefault("MYCRO_LOCAL_CACHE", "1")
if "/opt/trn_rl_repo" not in sys.path:
    sys.path.insert(0, "/opt/trn_rl_repo")

from contextlib import ExitStack

import numpy as np

B, T, D = 8, 4096, 2048
L = 127                 # x rows per main chunk (+1 carry row = K of 128)
NCHUNK = T // L         # 32 full chunks (ids 0..31)
TAIL = T - NCHUNK * L   # 32 trailing rows (chunk id 32)
DT = 512                # D tile width (one PSUM bank of fp32)
NT = D // DT            # 4 D tiles
GSZ = 4                 # chunks per DMA group (ids 1.. grouped)
N_CORES = 8
LTW = 128 + 128 + (TAIL + 1)  # packed weight tensor width

_compiled = {}


def _build_weights(decay_logit: np.ndarray):
    # Match the reference: decay = sigmoid(decay_logit) evaluated in fp32,
    # powers computed in fp64 from that fp32 value, rounded to fp32.
    logit = np.float64(np.asarray(decay_logit, dtype=np.float32))
    decay = np.float64(np.float32(1.0 / (1.0 + np.exp(-logit))))

    def lhs_t(rows, with_carry):
        # lhsT is [K, M]; out = lhsT.T @ rhs.
        # Output column m: m=0 is the carry-out (scan position rows-1),
        # m=1+t is scan position t.
        # Contraction p: with_carry -> p=0 is the carry row, p=1+s is x row s;
        # else p=s is x row s.
        pw = decay ** np.arange(rows + 1, dtype=np.float64)
        tri = np.zeros((rows, rows), np.float64)
        for s in range(rows):
            tri[s, s:] = pw[: rows - s]
        k = rows + 1 if with_carry else rows
        m = np.zeros((k, rows + 1), np.float64)
        if with_carry:
            m[0, 0] = pw[rows]          # carry -> carry-out
            m[1:, 0] = pw[rows - 1 :: -1]
            m[0, 1:] = pw[1:]           # carry -> position t
            m[1:, 1:] = tri
        else:
            m[:, 0] = pw[rows - 1 :: -1]
            m[:, 1:] = tri
        return m.astype(np.float32)

    lt_first = lhs_t(L, with_carry=False)   # [127, 128]
    lt_main = lhs_t(L, with_carry=True)     # [128, 128]
    lt_tail = lhs_t(TAIL, with_carry=True)  # [33, 33]

    packed = np.zeros((128, LTW), np.float32)
    packed[:127, 0:128] = lt_first
    packed[:, 128:256] = lt_main
    packed[: TAIL + 1, 256 : 256 + TAIL + 1] = lt_tail
    return packed


def _build_program():
    import concourse.bacc as bacc
    import concourse.mybir as mybir
    from concourse.tile import TileContext

    f32 = mybir.dt.float32
    f32r = mybir.dt.float32r
    nc = bacc.Bacc(trn_type="TRN2", target_bir_lowering=False, debug=False)

    x_d = nc.dram_tensor("x", [T, D], f32r, kind="ExternalInput")
    lt_d = nc.dram_tensor("lt_all", [128, LTW], f32r, kind="ExternalInput")
    y_d = nc.dram_tensor("y", [T, D], f32, kind="ExternalOutput")

    # group g covers chunk ids 1+GSZ*g .. min(GSZ*(g+1), 32); the last group's
    # final chunk is the 32-row tail, DMA'd separately into the same tile.
    groups = []
    k = 1
    while k <= NCHUNK:  # ids 1..32
        ids = list(range(k, min(k + GSZ, NCHUNK + 1)))
        groups.append(ids)
        k += GSZ
    chunk_rows = [L] * NCHUNK + [TAIL]

    with TileContext(nc) as tc, ExitStack() as ctx:
        const = ctx.enter_context(tc.tile_pool(name="const", bufs=1))
        lt = const.tile([128, LTW], f32r, name="lt")
        nc.sync.dma_start(lt[:, :], lt_d[:, :])
        lt_first = lt[0:L, 0:128]
        lt_main = lt[0:128, 128:256]
        lt_tail = lt[0 : TAIL + 1, 256 : 256 + TAIL + 1]

        x0p = ctx.enter_context(tc.tile_pool(name="x0p", bufs=1))
        y0p = ctx.enter_context(tc.tile_pool(name="y0p", bufs=1))
        xin_pool = ctx.enter_context(tc.tile_pool(name="xin", bufs=3))
        yout_pool = ctx.enter_context(tc.tile_pool(name="yout", bufs=2))
        ps_pool = ctx.enter_context(tc.tile_pool(name="ps", bufs=8, space="PSUM"))

        # ---- input DMAs (prefetch order; slot availability paces them) ----
        xmap = {}  # chunk id -> (tile, col_base)
        x0 = x0p.tile([L, D], f32r, name="xt0")
        nc.gpsimd.dma_start(x0[:, :], x_d[0:L, :])
        xmap[0] = (x0, 0)
        for g, ids in enumerate(groups):
            main_ids = [i for i in ids if chunk_rows[i] == L]
            nm = len(main_ids)
            xt = xin_pool.tile([128, GSZ * D], f32r, name=f"xg{g}", tag="xg")
            r0 = main_ids[0] * L
            src = x_d[r0 : r0 + nm * L, :].rearrange("(c p) n -> p c n", c=nm)
            dst = xt[1:128, 0 : nm * D].rearrange("p (c n) -> p c n", c=nm)
            nc.gpsimd.dma_start(dst, src)
            if nm < len(ids):  # tail chunk rides the last column range
                ci = len(ids) - 1
                nc.gpsimd.dma_start(
                    xt[1 : TAIL + 1, ci * D : ci * D + D],
                    x_d[NCHUNK * L : NCHUNK * L + TAIL, :],
                )
            for ci, i in enumerate(ids):
                xmap[i] = (xt, ci * D)

        # ---- compute + output ----
        ymap = {}
        y0 = y0p.tile([128, D], f32, name="yt0")
        ymap[0] = (y0, 0)
        for g, ids in enumerate(groups):
            yt = yout_pool.tile([128, GSZ * D], f32, name=f"yg{g}", tag="yg")
            for ci, i in enumerate(ids):
                ymap[i] = (yt, ci * D)

        for k in range(NCHUNK + 1):  # chunk ids 0..32
            rows = chunk_rows[k]
            lhsT = lt_first if k == 0 else (lt_tail if k == NCHUNK else lt_main)
            xt, xcb = xmap[k]
            yt, ycb = ymap[k]
            m = rows + 1  # psum partitions (row 0 = carry-out)
            for j in range(NT):
                ps = ps_pool.tile([m, DT], f32, name=f"ps{k}_{j}", tag="ps")
                nc.tensor.matmul(
                    ps[:, :],
                    lhsT,
                    xt[0 : lhsT.shape[0], xcb + j * DT : xcb + (j + 1) * DT],
                    start=True,
                    stop=True,
                )
                if k + 1 <= NCHUNK:
                    nxt, ncb = xmap[k + 1]
                    # carry row for chunk k+1, on ScalarE straight from PSUM
                    nc.scalar.copy(
                        nxt[0:1, ncb + j * DT : ncb + (j + 1) * DT],
                        ps[0:1, :],
                    )
                nc.vector.tensor_copy(yt[0:m, ycb + j * DT : ycb + (j + 1) * DT], ps[:, :])
            # output DMA once a tile's last chunk is done
            if k == 0:
                nc.gpsimd.dma_start(y_d[0:L, :], y0[1 : L + 1, :])
            elif k == xmapped_last(groups, k):
                g = (k - 1) // GSZ
                ids = groups[g]
                main_ids = [i for i in ids if chunk_rows[i] == L]
                nm = len(main_ids)
                r0 = main_ids[0] * L
                src = yt[1:128, 0 : nm * D].rearrange("p (c n) -> p c n", c=nm)
                dst = y_d[r0 : r0 + nm * L, :].rearrange("(c p) n -> p c n", c=nm)
                nc.gpsimd.dma_start(dst, src)
                if nm < len(ids):
                    ci = len(ids) - 1
                    nc.gpsimd.dma_start(
                        y_d[NCHUNK * L : NCHUNK * L + TAIL, :],
                        yt[1 : TAIL + 1, ci * D : ci * D + D],
                    )

    nc.finalize()
    return nc


def xmapped_last(groups, k):
    for ids in groups:
        if k in ids:
            return ids[-1]
    return -1


def _get_program():
    if "nc" not in _compiled:
        _compiled["nc"] = _build_program()
    return _compiled["nc"]


def _install_profile_hook():
    """The container's `antenv` lacks `axon_hooks`, so NTFF profiling under
    axon degrades silently. Synthesize the module and install the ctypes hook
    from trn_agent_boot (same thing boot() would have done)."""
    if "antenv.axon_hooks" in sys.modules:
        return
    import types

    import antenv

    mod = types.ModuleType("antenv.axon_hooks")
    state = {"hook": None}
    mod.set_axon_ntff_profile_hook = lambda h: state.__setitem__("hook", h)
    mod.get_axon_ntff_profile_hook = lambda: state["hook"]
    sys.modules["antenv.axon_hooks"] = mod
    antenv.axon_hooks = mod

    from trn_agent_boot.trn_boot import _ntff_profile_via_ctypes

    mod.set_axon_ntff_profile_hook(
        _ntff_profile_via_ctypes("/opt/axon/libaxon_pjrt.so")
    )

    # no S3 in this container — keep artifacts local
    from concourse import bass_utils

    bass_utils.upload_artifacts = lambda tmpdir: tmpdir


def _run(x, decay_logit, trace=False):
    from concourse.bass_utils import run_bass_kernel_spmd

    if trace:
        _install_profile_hook()

    x = np.ascontiguousarray(np.asarray(x, dtype=np.float32))
    assert x.shape == (B, T, D), x.shape
    lt_all = _build_weights(decay_logit)

    nc = _get_program()
    in_maps = [
        {"x": np.ascontiguousarray(x[b]), "lt_all": lt_all} for b in range(N_CORES)
    ]
    res = run_bass_kernel_spmd(
        nc,
        in_maps,
        core_ids=list(range(N_CORES)),
        trace=trace,
        trace_cores=[0] if trace else None,
    )
    y = np.stack([res.results[b]["y"] for b in range(N_CORES)], axis=0)
    return y, res


def kernel(x, decay_logit):
    y, _ = _run(x, decay_logit, trace=False)
    return y


def kernel_traced(x, decay_logit):
    """Like kernel() but returns (y, BassKernelResults) with NTFF profile."""
    return _run(x, decay_logit, trace=True)


# revision 11
# speedup vs baseline: 1.0511x; 1.0511x over previous
"""EMA recurrence kernel for Trainium2 (8 NeuronCores, batch-parallel).

Computes c[b,t,d] = x[b,t,d] + decay * c[b,t-1,d]  (decay = sigmoid(decay_logit))
for x of shape (8, 4096, 2048) fp32, as a blocked scan:

  - T is split into chunks of L=127 rows. Within a chunk the scan is a
    triangular matmul: out[t,d] = sum_{s<=t} decay^(t-s) x[s,d].
  - The cross-chunk carry (c at the last row of the previous chunk) is folded
    into the same matmul as an extra contraction row whose weight column is
    decay^(t+1) — so each chunk is ONE matmul per 512-wide D tile.  Matmuls
    run in float32r (single-pass fp32 PE mode, ~1e-4 rel err) instead of the
    2-pass exact fp32 mode, which would make PE the bottleneck.
  - Layout: the carry input row lives at SBUF partition 0 (x rows at
    partitions 1..127), and the matmul's output columns are permuted so that
    PSUM partition 0 holds the chunk's LAST scan position (the next carry)
    and partitions 1..127 hold scan positions 0..126.  All compute-engine
    access patterns therefore start at partition 0 (the BIR verifier rejects
    engine APs starting at non-32-aligned partitions); only DMA (which has
    no partition-alignment restriction) touches rows 1..127.  Chunk 0 has no
    carry: it uses its own weight matrix with x rows at partitions 0..126.
  - Carry copies run on ScalarE straight from PSUM (so the PE chain does not
    wait on VectorE's output copies); output copies run on VectorE.
  - DMA: chunks are grouped 4-per-dma_start (~4 MB contiguous-per-row 3D APs)
    on the SWDGE/gpsimd path — the only path that sprays descriptors across
    all 16 SDMA engines (HWDGE serializes everything onto one engine).
  - Batch b is sharded across the 8 cores (one b per core).
"""

import os
import sys

os.environ.setdefault("MYCRO_LOCAL_CACHE", "1")
if "/opt/trn_rl_repo" not in sys.path:
    sys.path.insert(0, "/opt/trn_rl_repo")

from contextlib import ExitStack

import numpy as np

B, T, D = 8, 4096, 2048
L = 127                 # x rows per main chunk (+1 carry row = K of 128)
NCHUNK = T // L         # 32 full chunks (ids 0..31)
TAIL = T - NCHUNK * L   # 32 trailing rows (chunk id 32)
DT = 512                # D tile width (one PSUM bank of fp32)
NT = D // DT            # 4 D tiles
GSZ = 4                 # chunks per DMA group (ids 1.. grouped)
N_CORES = 8
LTW = 128 + 128 + (TAIL + 1)  # packed weight tensor width

_compiled = {}


def _build_weights(decay_logit: np.ndarray):
    # Match the reference: decay = sigmoid(decay_logit) evaluated in fp32,
    # powers computed in fp64 from that fp32 value, rounded to fp32.
    logit = np.float64(np.asarray(decay_logit, dtype=np.float32))
    decay = np.float64(np.float32(1.0 / (1.0 + np.exp(-logit))))

    def lhs_t(rows, with_carry):
        # lhsT is [K, M]; out = lhsT.T @ rhs.
        # Output column m: m=0 is the carry-out (scan position rows-1),
        # m=1+t is scan position t.
        # Contraction p: with_carry -> p=0 is the carry row, p=1+s is x row s;
        # else p=s is x row s.
        pw = decay ** np.arange(rows + 1, dtype=np.float64)
        tri = np.zeros((rows, rows), np.float64)
        for s in range(rows):
            tri[s, s:] = pw[: rows - s]
        k = rows + 1 if with_carry else rows
        m = np.zeros((k, rows + 1), np.float64)
        if with_carry:
            m[0, 0] = pw[rows]          # carry -> carry-out
            m[1:, 0] = pw[rows - 1 :: -1]
            m[0, 1:] = pw[1:]           # carry -> position t
            m[1:, 1:] = tri
        else:
            m[:, 0] = pw[rows - 1 :: -1]
            m[:, 1:] = tri
        return m.astype(np.float32)

    lt_first = lhs_t(L, with_carry=False)   # [127, 128]
    lt_main = lhs_t(L, with_carry=True)     # [128, 128]
    lt_tail = lhs_t(TAIL, with_carry=True)  # [33, 33]

    packed = np.zeros((128, LTW), np.float32)
    packed[:127, 0:128] = lt_first
    packed[:, 128:256] = lt_main
    packed[: TAIL + 1, 256 : 256 + TAIL + 1] = lt_tail
    return packed


def _build_program():
    import concourse.bacc as bacc
    import concourse.mybir as mybir
    from concourse.tile import TileContext

    f32 = mybir.dt.float32
    f32r = mybir.dt.float32r
    nc = bacc.Bacc(trn_type="TRN2", target_bir_lowering=False, debug=False)

    x_d = nc.dram_tensor("x", [T, D], f32r, kind="ExternalInput")
    lt_d = nc.dram_tensor("lt_all", [128, LTW], f32r, kind="ExternalInput")
    y_d = nc.dram_tensor("y", [T, D], f32, kind="ExternalOutput")

    # group g covers chunk ids 1+GSZ*g .. min(GSZ*(g+1), 32); the last group's
    # final chunk is the 32-row tail, DMA'd separately into the same tile.
    groups = []
    k = 1
    while k <= NCHUNK:  # ids 1..32
        ids = list(range(k, min(k + GSZ, NCHUNK + 1)))
        groups.append(ids)
        k += GSZ
    chunk_rows = [L] * NCHUNK + [TAIL]

    with TileContext(nc) as tc, ExitStack() as ctx:
        const = ctx.enter_context(tc.tile_pool(name="const", bufs=1))
        lt = const.tile([128, LTW], f32r, name="lt")
        nc.sync.dma_start(lt[:, :], lt_d[:, :])
        lt_first = lt[0:L, 0:128]
        lt_main = lt[0:128, 128:256]
        lt_tail = lt[0 : TAIL + 1, 256 : 256 + TAIL + 1]

        x0p = ctx.enter_context(tc.tile_pool(name="x0p", bufs=1))
        y0p = ctx.enter_context(tc.tile_pool(name="y0p", bufs=1))
        xin_pool = ctx.enter_context(tc.tile_pool(name="xin", bufs=3))
        yout_pool = ctx.enter_context(tc.tile_pool(name="yout", bufs=2))
        ps_pool = ctx.enter_context(tc.tile_pool(name="ps", bufs=8, space="PSUM"))

        xmap = {}  # chunk id -> (tile, col_base)
        ymap = {}

        def emit_in_dma(g):
            ids = groups[g]
            main_ids = [i for i in ids if chunk_rows[i] == L]
            nm = len(main_ids)
            xt = xin_pool.tile([128, GSZ * D], f32r, name=f"xg{g}", tag="xg")
            r0 = main_ids[0] * L
            src = x_d[r0 : r0 + nm * L, :].rearrange("(c p) n -> p c n", c=nm)
            dst = xt[1:128, 0 : nm * D].rearrange("p (c n) -> p c n", c=nm)
            nc.gpsimd.dma_start(dst, src)
            if nm < len(ids):  # tail chunk rides the last column range
                ci = len(ids) - 1
                nc.gpsimd.dma_start(
                    xt[1 : TAIL + 1, ci * D : ci * D + D],
                    x_d[NCHUNK * L : NCHUNK * L + TAIL, :],
                )
            for ci, i in enumerate(ids):
                xmap[i] = (xt, ci * D)

        def emit_out_dma(g):
            ids = groups[g]
            yt, _ = ymap[ids[0]]
            main_ids = [i for i in ids if chunk_rows[i] == L]
            nm = len(main_ids)
            r0 = main_ids[0] * L
            src = yt[1:128, 0 : nm * D].rearrange("p (c n) -> p c n", c=nm)
            dst = y_d[r0 : r0 + nm * L, :].rearrange("(c p) n -> p c n", c=nm)
            nc.gpsimd.dma_start(dst, src)
            if nm < len(ids):
                ci = len(ids) - 1
                nc.gpsimd.dma_start(
                    y_d[NCHUNK * L : NCHUNK * L + TAIL, :],
                    yt[1 : TAIL + 1, ci * D : ci * D + D],
                )

        def compute_chunk(k):
            rows = chunk_rows[k]
            lhsT = lt_first if k == 0 else (lt_tail if k == NCHUNK else lt_main)
            xt, xcb = xmap[k]
            yt, ycb = ymap[k]
            m = rows + 1  # psum partitions (row 0 = carry-out)
            for j in range(NT):
                ps = ps_pool.tile([m, DT], f32, name=f"ps{k}_{j}", tag="ps")
                nc.tensor.matmul(
                    ps[:, :],
                    lhsT,
                    xt[0 : lhsT.shape[0], xcb + j * DT : xcb + (j + 1) * DT],
                    start=True,
                    stop=True,
                )
                if k + 1 <= NCHUNK:
                    nxt, ncb = xmap[k + 1]
                    # carry row for chunk k+1, on ScalarE straight from PSUM
                    nc.scalar.copy(
                        nxt[0:1, ncb + j * DT : ncb + (j + 1) * DT],
                        ps[0:1, :],
                    )
                nc.vector.tensor_copy(
                    yt[0:m, ycb + j * DT : ycb + (j + 1) * DT], ps[:, :]
                )

        # ---- emission order ----
        # GpSimd's SWDGE issue queue is strict in-order, so no DMA may be
        # emitted whose semaphore wait will stall later DMAs behind it
        # (head-of-line blocking). Out-DMAs are therefore emitted one group
        # LATE (their compute finished a whole group ago) and in-DMAs two
        # groups EARLY (their slot was released a group ago).
        x0 = x0p.tile([L, D], f32r, name="xt0")
        nc.gpsimd.dma_start(x0[:, :], x_d[0:L, :])
        xmap[0] = (x0, 0)
        emit_in_dma(0)
        emit_in_dma(1)

        y0 = y0p.tile([128, D], f32, name="yt0")
        ymap[0] = (y0, 0)
        compute_chunk(0)

        for g in range(len(groups)):
            if g + 2 < len(groups):
                emit_in_dma(g + 2)
            if g == 0:
                nc.gpsimd.dma_start(y_d[0:L, :], y0[1 : L + 1, :])
            else:
                emit_out_dma(g - 1)
            yt = yout_pool.tile([128, GSZ * D], f32, name=f"yg{g}", tag="yg")
            for ci, i in enumerate(groups[g]):
                ymap[i] = (yt, ci * D)
            for k in groups[g]:
                compute_chunk(k)
        emit_out_dma(len(groups) - 1)

    nc.finalize()
    return nc


def _get_program():
    if "nc" not in _compiled:
        _compiled["nc"] = _build_program()
    return _compiled["nc"]


def _install_profile_hook():
    """The container's `antenv` lacks `axon_hooks`, so NTFF profiling under
    axon degrades silently. Synthesize the module and install the ctypes hook
    from trn_agent_boot (same thing boot() would have done)."""
    if "antenv.axon_hooks" in sys.modules:
        return
    import types

    import antenv

    mod = types.ModuleType("antenv.axon_hooks")
    state = {"hook": None}
    mod.set_axon_ntff_profile_hook = lambda h: state.__setitem__("hook", h)
    mod.get_axon_ntff_profile_hook = lambda: state["hook"]
    sys.modules["antenv.axon_hooks"] = mod
    antenv.axon_hooks = mod

    from trn_agent_boot.trn_boot import _ntff_profile_via_ctypes

    mod.set_axon_ntff_profile_hook(
        _ntff_profile_via_ctypes("/opt/axon/libaxon_pjrt.so")
    )

    # no S3 in this container — keep artifacts local
    from concourse import bass_utils

    bass_utils.upload_artifacts = lambda tmpdir: tmpdir


def _run(x, decay_logit, trace=False):
    from concourse.bass_utils import run_bass_kernel_spmd

    if trace:
        _install_profile_hook()

    x = np.ascontiguousarray(np.asarray(x, dtype=np.float32))
    assert x.shape == (B, T, D), x.shape
    lt_all = _build_weights(decay_logit)

    nc = _get_program()
    in_maps = [
        {"x": np.ascontiguousarray(x[b]), "lt_all": lt_all} for b in range(N_CORES)
    ]
    res = run_bass_kernel_spmd(
        nc,
        in_maps,
        core_ids=list(range(N_CORES)),
        trace=trace,
        trace_cores=[0] if trace else None,
    )
    y = np.stack([res.results[b]["y"] for b in range(N_CORES)], axis=0)
    return y, res


def kernel(x, decay_logit):
    y, _ = _run(x, decay_logit, trace=False)
    return y


def kernel_traced(x, decay_logit):
    """Like kernel() but returns (y, BassKernelResults) with NTFF profile."""
    return _run(x, decay_logit, trace=True)


# revision 12
# speedup vs baseline: 3.0211x; 2.8742x over previous
"""EMA recurrence kernel for Trainium2 (8 NeuronCores, batch-parallel).

Computes c[b,t,d] = x[b,t,d] + decay * c[b,t-1,d]  (decay = sigmoid(decay_logit))
for x of shape (8, 4096, 2048) fp32, as a blocked scan:

  - T is split into chunks of L=127 rows. Within a chunk the scan is a
    triangular matmul: out[t,d] = sum_{s<=t} decay^(t-s) x[s,d].
  - The cross-chunk carry (c at the last row of the previous chunk) is folded
    into the same matmul as an extra contraction row whose weight column is
    decay^(t+1) — so each chunk is ONE matmul per 512-wide D tile.  Matmuls
    run in float32r (single-pass fp32 PE mode, ~1e-4 rel err) instead of the
    2-pass exact fp32 mode, which would make PE the bottleneck.
  - Layout: the carry input row lives at SBUF partition 0 (x rows at
    partitions 1..127), and the matmul's output columns are permuted so that
    PSUM partition 0 holds the chunk's LAST scan position (the next carry)
    and partitions 1..127 hold scan positions 0..126.  All compute-engine
    access patterns therefore start at partition 0 (the BIR verifier rejects
    engine APs starting at non-32-aligned partitions); only DMA (which has
    no partition-alignment restriction) touches rows 1..127.  Chunk 0 has no
    carry: it uses its own weight matrix with x rows at partitions 0..126.
  - Carry copies run on ScalarE straight from PSUM (so the PE chain does not
    wait on VectorE's output copies); output copies run on VectorE.
  - DMA: chunks are grouped 4-per-dma_start (~4 MB contiguous-per-row 3D APs)
    on the SWDGE/gpsimd path — the only path that sprays descriptors across
    all 16 SDMA engines (HWDGE serializes everything onto one engine).
  - Batch b is sharded across the 8 cores (one b per core).
"""

import os
import sys

os.environ.setdefault("MYCRO_LOCAL_CACHE", "1")
if "/opt/trn_rl_repo" not in sys.path:
    sys.path.insert(0, "/opt/trn_rl_repo")

from contextlib import ExitStack

import numpy as np

B, T, D = 8, 4096, 2048
L = 127                 # x rows per main chunk (+1 carry row = K of 128)
NCHUNK = T // L         # 32 full chunks (ids 0..31)
TAIL = T - NCHUNK * L   # 32 trailing rows (chunk id 32)
DT = 512                # D tile width (one PSUM bank of fp32)
NT = D // DT            # 4 D tiles
GSZ = 4                 # chunks per DMA group (ids 1.. grouped)
N_CORES = 8
LTW = 128 + 128 + (TAIL + 1)  # packed weight tensor width

_compiled = {}


def _build_weights(decay_logit: np.ndarray):
    # Match the reference: decay = sigmoid(decay_logit) evaluated in fp32,
    # powers computed in fp64 from that fp32 value, rounded to fp32.
    logit = np.float64(np.asarray(decay_logit, dtype=np.float32))
    decay = np.float64(np.float32(1.0 / (1.0 + np.exp(-logit))))

    def lhs_t(rows, with_carry):
        # lhsT is [K, M]; out = lhsT.T @ rhs.
        # Output column m: m=0 is the carry-out (scan position rows-1),
        # m=1+t is scan position t.
        # Contraction p: with_carry -> p=0 is the carry row, p=1+s is x row s;
        # else p=s is x row s.
        pw = decay ** np.arange(rows + 1, dtype=np.float64)
        tri = np.zeros((rows, rows), np.float64)
        for s in range(rows):
            tri[s, s:] = pw[: rows - s]
        k = rows + 1 if with_carry else rows
        m = np.zeros((k, rows + 1), np.float64)
        if with_carry:
            m[0, 0] = pw[rows]          # carry -> carry-out
            m[1:, 0] = pw[rows - 1 :: -1]
            m[0, 1:] = pw[1:]           # carry -> position t
            m[1:, 1:] = tri
        else:
            m[:, 0] = pw[rows - 1 :: -1]
            m[:, 1:] = tri
        return m.astype(np.float32)

    lt_first = lhs_t(L, with_carry=False)   # [127, 128]
    lt_main = lhs_t(L, with_carry=True)     # [128, 128]
    lt_tail = lhs_t(TAIL, with_carry=True)  # [33, 33]

    packed = np.zeros((128, LTW), np.float32)
    packed[:127, 0:128] = lt_first
    packed[:, 128:256] = lt_main
    packed[: TAIL + 1, 256 : 256 + TAIL + 1] = lt_tail
    return packed


def _build_program():
    import concourse.bacc as bacc
    import concourse.mybir as mybir
    from concourse.tile import TileContext

    f32 = mybir.dt.float32
    f32r = mybir.dt.float32r
    nc = bacc.Bacc(trn_type="TRN2", target_bir_lowering=False, debug=False)

    x_d = nc.dram_tensor("x", [T, D], f32r, kind="ExternalInput")
    lt_d = nc.dram_tensor("lt_all", [128, LTW], f32r, kind="ExternalInput")
    y_d = nc.dram_tensor("y", [T, D], f32, kind="ExternalOutput")

    # group g covers chunk ids 1+GSZ*g .. min(GSZ*(g+1), 32); the last group's
    # final chunk is the 32-row tail, DMA'd separately into the same tile.
    groups = []
    k = 1
    while k <= NCHUNK:  # ids 1..32
        ids = list(range(k, min(k + GSZ, NCHUNK + 1)))
        groups.append(ids)
        k += GSZ
    chunk_rows = [L] * NCHUNK + [TAIL]

    with TileContext(nc) as tc, ExitStack() as ctx:
        const = ctx.enter_context(tc.tile_pool(name="const", bufs=1))
        lt = const.tile([128, LTW], f32r, name="lt")
        nc.sync.dma_start(lt[:, :], lt_d[:, :])
        lt_first = lt[0:L, 0:128]
        lt_main = lt[0:128, 128:256]
        lt_tail = lt[0 : TAIL + 1, 256 : 256 + TAIL + 1]

        x0p = ctx.enter_context(tc.tile_pool(name="x0p", bufs=1))
        y0p = ctx.enter_context(tc.tile_pool(name="y0p", bufs=1))
        xin_pool = ctx.enter_context(tc.tile_pool(name="xin", bufs=3))
        yout_pool = ctx.enter_context(tc.tile_pool(name="yout", bufs=2))
        ps_pool = ctx.enter_context(tc.tile_pool(name="ps", bufs=8, space="PSUM"))

        xmap = {}  # chunk id -> (tile, col_base)
        ymap = {}

        def emit_in_dma(g):
            # per-chunk 2D dma_starts: only plain [partitions, row] APs get
            # the SWDGE 16-engine descriptor spray (3D APs land on 1 engine)
            ids = groups[g]
            xt = xin_pool.tile([128, GSZ * D], f32r, name=f"xg{g}", tag="xg")
            for ci, i in enumerate(ids):
                rows = chunk_rows[i]
                nc.gpsimd.dma_start(
                    xt[1 : rows + 1, ci * D : ci * D + D],
                    x_d[i * L : i * L + rows, :],
                )
                xmap[i] = (xt, ci * D)

        def emit_out_dma(g):
            ids = groups[g]
            yt, _ = ymap[ids[0]]
            for ci, i in enumerate(ids):
                rows = chunk_rows[i]
                nc.gpsimd.dma_start(
                    y_d[i * L : i * L + rows, :],
                    yt[1 : rows + 1, ci * D : ci * D + D],
                )

        def compute_chunk(k):
            rows = chunk_rows[k]
            lhsT = lt_first if k == 0 else (lt_tail if k == NCHUNK else lt_main)
            xt, xcb = xmap[k]
            yt, ycb = ymap[k]
            m = rows + 1  # psum partitions (row 0 = carry-out)
            for j in range(NT):
                ps = ps_pool.tile([m, DT], f32, name=f"ps{k}_{j}", tag="ps")
                nc.tensor.matmul(
                    ps[:, :],
                    lhsT,
                    xt[0 : lhsT.shape[0], xcb + j * DT : xcb + (j + 1) * DT],
                    start=True,
                    stop=True,
                )
                if k + 1 <= NCHUNK:
                    nxt, ncb = xmap[k + 1]
                    # carry row for chunk k+1, on ScalarE straight from PSUM
                    nc.scalar.copy(
                        nxt[0:1, ncb + j * DT : ncb + (j + 1) * DT],
                        ps[0:1, :],
                    )
                nc.vector.tensor_copy(
                    yt[0:m, ycb + j * DT : ycb + (j + 1) * DT], ps[:, :]
                )

        # ---- emission order ----
        # GpSimd's SWDGE issue queue is strict in-order, so no DMA may be
        # emitted whose semaphore wait will stall later DMAs behind it
        # (head-of-line blocking). Out-DMAs are therefore emitted one group
        # LATE (their compute finished a whole group ago) and in-DMAs two
        # groups EARLY (their slot was released a group ago).
        x0 = x0p.tile([L, D], f32r, name="xt0")
        nc.gpsimd.dma_start(x0[:, :], x_d[0:L, :])
        xmap[0] = (x0, 0)
        emit_in_dma(0)
        emit_in_dma(1)

        y0 = y0p.tile([128, D], f32, name="yt0")
        ymap[0] = (y0, 0)
        compute_chunk(0)

        for g in range(len(groups)):
            if g + 2 < len(groups):
                emit_in_dma(g + 2)
            if g == 0:
                nc.gpsimd.dma_start(y_d[0:L, :], y0[1 : L + 1, :])
            else:
                emit_out_dma(g - 1)
            yt = yout_pool.tile([128, GSZ * D], f32, name=f"yg{g}", tag="yg")
            for ci, i in enumerate(groups[g]):
                ymap[i] = (yt, ci * D)
            for k in groups[g]:
                compute_chunk(k)
        emit_out_dma(len(groups) - 1)

    nc.finalize()
    return nc


def _get_program():
    if "nc" not in _compiled:
        _compiled["nc"] = _build_program()
    return _compiled["nc"]


def _install_profile_hook():
    """The container's `antenv` lacks `axon_hooks`, so NTFF profiling under
    axon degrades silently. Synthesize the module and install the ctypes hook
    from trn_agent_boot (same thing boot() would have done)."""
    if "antenv.axon_hooks" in sys.modules:
        return
    import types

    import antenv

    mod = types.ModuleType("antenv.axon_hooks")
    state = {"hook": None}
    mod.set_axon_ntff_profile_hook = lambda h: state.__setitem__("hook", h)
    mod.get_axon_ntff_profile_hook = lambda: state["hook"]
    sys.modules["antenv.axon_hooks"] = mod
    antenv.axon_hooks = mod

    from trn_agent_boot.trn_boot import _ntff_profile_via_ctypes

    mod.set_axon_ntff_profile_hook(
        _ntff_profile_via_ctypes("/opt/axon/libaxon_pjrt.so")
    )

    # no S3 in this container — keep artifacts local
    from concourse import bass_utils

    bass_utils.upload_artifacts = lambda tmpdir: tmpdir


def _run(x, decay_logit, trace=False):
    from concourse.bass_utils import run_bass_kernel_spmd

    if trace:
        _install_profile_hook()

    x = np.ascontiguousarray(np.asarray(x, dtype=np.float32))
    assert x.shape == (B, T, D), x.shape
    lt_all = _build_weights(decay_logit)

    nc = _get_program()
    in_maps = [
        {"x": np.ascontiguousarray(x[b]), "lt_all": lt_all} for b in range(N_CORES)
    ]
    res = run_bass_kernel_spmd(
        nc,
        in_maps,
        core_ids=list(range(N_CORES)),
        trace=trace,
        trace_cores=[0] if trace else None,
    )
    y = np.stack([res.results[b]["y"] for b in range(N_CORES)], axis=0)
    return y, res


def kernel(x, decay_logit):
    y, _ = _run(x, decay_logit, trace=False)
    return y


def kernel_traced(x, decay_logit):
    """Like kernel() but returns (y, BassKernelResults) with NTFF profile."""
    return _run(x, decay_logit, trace=True)


# revision 15
# speedup vs baseline: 3.0592x; 1.0126x over previous
"""EMA recurrence kernel for Trainium2 (8 NeuronCores, batch-parallel).

Computes c[b,t,d] = x[b,t,d] + decay * c[b,t-1,d]  (decay = sigmoid(decay_logit))
for x of shape (8, 4096, 2048) fp32, as a blocked scan:

  - T is split into chunks of L=127 rows. Within a chunk the scan is a
    triangular matmul: out[t,d] = sum_{s<=t} decay^(t-s) x[s,d].
  - The cross-chunk carry (c at the last row of the previous chunk) is folded
    into the same matmul as an extra contraction row whose weight column is
    decay^(t+1) — so each chunk is ONE matmul per 512-wide D tile.  Matmuls
    run in float32r (single-pass fp32 PE mode, ~1e-4 rel err) instead of the
    2-pass exact fp32 mode, which would make PE the bottleneck.
  - Layout: the carry input row lives at SBUF partition 0 (x rows at
    partitions 1..127), and the matmul's output columns are permuted so that
    PSUM partition 0 holds the chunk's LAST scan position (the next carry)
    and partitions 1..127 hold scan positions 0..126.  All compute-engine
    access patterns therefore start at partition 0 (the BIR verifier rejects
    engine APs starting at non-32-aligned partitions); only DMA (which has
    no partition-alignment restriction) touches rows 1..127.  Chunk 0 has no
    carry: it uses its own weight matrix with x rows at partitions 0..126.
  - Carry copies run on ScalarE straight from PSUM (so the PE chain does not
    wait on VectorE's output copies); output copies run on VectorE.
  - DMA: chunks are grouped 4-per-dma_start (~4 MB contiguous-per-row 3D APs)
    on the SWDGE/gpsimd path — the only path that sprays descriptors across
    all 16 SDMA engines (HWDGE serializes everything onto one engine).
  - Batch b is sharded across the 8 cores (one b per core).
"""

import os
import sys

os.environ.setdefault("MYCRO_LOCAL_CACHE", "1")
if "/opt/trn_rl_repo" not in sys.path:
    sys.path.insert(0, "/opt/trn_rl_repo")

from contextlib import ExitStack

import numpy as np

B, T, D = 8, 4096, 2048
L = 127                 # x rows per main chunk (+1 carry row = K of 128)
NCHUNK = T // L         # 32 full chunks (ids 0..31)
TAIL = T - NCHUNK * L   # 32 trailing rows (chunk id 32)
DT = 512                # D tile width (one PSUM bank of fp32)
NT = D // DT            # 4 D tiles
GSZ = 2                 # chunks per SBUF tile group
N_CORES = 8
LTW = 128 + (TAIL + 1) + D  # weights + a zero row for chunk 0's carry

_compiled = {}


def _build_weights(decay_logit: np.ndarray):
    # Match the reference: decay = sigmoid(decay_logit) evaluated in fp32,
    # powers computed in fp64 from that fp32 value, rounded to fp32.
    logit = np.float64(np.asarray(decay_logit, dtype=np.float32))
    decay = np.float64(np.float32(1.0 / (1.0 + np.exp(-logit))))

    def lhs_t(rows, with_carry):
        # lhsT is [K, M]; out = lhsT.T @ rhs.
        # Output column m: m=0 is the carry-out (scan position rows-1),
        # m=1+t is scan position t.
        # Contraction p: with_carry -> p=0 is the carry row, p=1+s is x row s;
        # else p=s is x row s.
        pw = decay ** np.arange(rows + 1, dtype=np.float64)
        tri = np.zeros((rows, rows), np.float64)
        for s in range(rows):
            tri[s, s:] = pw[: rows - s]
        k = rows + 1 if with_carry else rows
        m = np.zeros((k, rows + 1), np.float64)
        if with_carry:
            m[0, 0] = pw[rows]          # carry -> carry-out
            m[1:, 0] = pw[rows - 1 :: -1]
            m[0, 1:] = pw[1:]           # carry -> position t
            m[1:, 1:] = tri
        else:
            m[:, 0] = pw[rows - 1 :: -1]
            m[:, 1:] = tri
        return m.astype(np.float32)

    lt_main = lhs_t(L, with_carry=True)     # [128, 128]
    lt_tail = lhs_t(TAIL, with_carry=True)  # [33, 33]

    packed = np.zeros((128, LTW), np.float32)
    packed[:, 0:128] = lt_main
    packed[: TAIL + 1, 128 : 128 + TAIL + 1] = lt_tail
    # columns 128+TAIL+1 .. end stay zero: the initial carry row for chunk 0
    return packed


def _build_program():
    import concourse.bacc as bacc
    import concourse.mybir as mybir
    from concourse.tile import TileContext

    f32 = mybir.dt.float32
    f32r = mybir.dt.float32r
    nc = bacc.Bacc(trn_type="TRN2", target_bir_lowering=False, debug=False)

    x_d = nc.dram_tensor("x", [T, D], f32r, kind="ExternalInput")
    lt_d = nc.dram_tensor("lt_all", [128, LTW], f32r, kind="ExternalInput")
    y_d = nc.dram_tensor("y", [T, D], f32, kind="ExternalOutput")

    # group g covers chunk ids GSZ*g .. min(GSZ*(g+1), 32)
    groups = []
    k = 0
    while k <= NCHUNK:  # ids 0..32
        ids = list(range(k, min(k + GSZ, NCHUNK + 1)))
        groups.append(ids)
        k += GSZ
    chunk_rows = [L] * NCHUNK + [TAIL]

    with TileContext(nc) as tc, ExitStack() as ctx:
        const = ctx.enter_context(tc.tile_pool(name="const", bufs=1))
        lt = const.tile([128, LTW], f32r, name="lt")
        nc.sync.dma_start(lt[:, :], lt_d[:, :])
        lt_main = lt[0:128, 0:128]
        lt_tail = lt[0 : TAIL + 1, 128 : 128 + TAIL + 1]
        zrow = lt[0:1, 128 + TAIL + 1 : 128 + TAIL + 1 + D]

        xin_pool = ctx.enter_context(tc.tile_pool(name="xin", bufs=6))
        yout_pool = ctx.enter_context(tc.tile_pool(name="yout", bufs=4))
        ps_pool = ctx.enter_context(tc.tile_pool(name="ps", bufs=8, space="PSUM"))

        xmap = {}  # chunk id -> (tile, col_base)
        ymap = {}

        def emit_in_dma(g):
            # per-chunk 2D dma_starts: only plain [partitions, row] APs get
            # the SWDGE 16-engine descriptor spray (3D APs land on 1 engine)
            ids = groups[g]
            xt = xin_pool.tile([128, GSZ * D], f32r, name=f"xg{g}", tag="xg")
            for ci, i in enumerate(ids):
                rows = chunk_rows[i]
                nc.gpsimd.dma_start(
                    xt[1 : rows + 1, ci * D : ci * D + D],
                    x_d[i * L : i * L + rows, :],
                )
                xmap[i] = (xt, ci * D)

        def emit_out_dma(g):
            ids = groups[g]
            yt, _ = ymap[ids[0]]
            for ci, i in enumerate(ids):
                rows = chunk_rows[i]
                nc.gpsimd.dma_start(
                    y_d[i * L : i * L + rows, :],
                    yt[1 : rows + 1, ci * D : ci * D + D],
                )

        def compute_chunk(k):
            rows = chunk_rows[k]
            lhsT = lt_tail if k == NCHUNK else lt_main
            xt, xcb = xmap[k]
            yt, ycb = ymap[k]
            m = rows + 1  # psum partitions (row 0 = carry-out)
            for j in range(NT):
                ps = ps_pool.tile([m, DT], f32, name=f"ps{k}_{j}", tag="ps")
                nc.tensor.matmul(
                    ps[:, :],
                    lhsT,
                    xt[0 : lhsT.shape[0], xcb + j * DT : xcb + (j + 1) * DT],
                    start=True,
                    stop=True,
                )
                if k + 1 <= NCHUNK:
                    nxt, ncb = xmap[k + 1]
                    # carry row for chunk k+1, on ScalarE straight from PSUM
                    nc.scalar.copy(
                        nxt[0:1, ncb + j * DT : ncb + (j + 1) * DT],
                        ps[0:1, :],
                    )
                nc.vector.tensor_copy(
                    yt[0:m, ycb + j * DT : ycb + (j + 1) * DT], ps[:, :]
                )

        # ---- emission order ----
        # GpSimd's SWDGE issue queue is strict in-order, so no DMA may be
        # emitted whose semaphore wait will stall later DMAs behind it
        # (head-of-line blocking). Out-DMAs are therefore emitted one group
        # LATE (their compute finished a whole group ago) and in-DMAs three
        # groups EARLY (their slot was released a group ago).
        emit_in_dma(0)
        nc.scalar.copy(xmap[0][0][0:1, 0:D], zrow)  # chunk 0 carry = 0
        emit_in_dma(1)
        emit_in_dma(2)

        for g in range(len(groups)):
            if g + 3 < len(groups):
                emit_in_dma(g + 3)
            if g >= 1:
                emit_out_dma(g - 1)
            yt = yout_pool.tile([128, GSZ * D], f32, name=f"yg{g}", tag="yg")
            for ci, i in enumerate(groups[g]):
                ymap[i] = (yt, ci * D)
            for k in groups[g]:
                compute_chunk(k)
        emit_out_dma(len(groups) - 1)

    nc.finalize()
    return nc


def _get_program():
    if "nc" not in _compiled:
        _compiled["nc"] = _build_program()
    return _compiled["nc"]


def _install_profile_hook():
    """The container's `antenv` lacks `axon_hooks`, so NTFF profiling under
    axon degrades silently. Synthesize the module and install the ctypes hook
    from trn_agent_boot (same thing boot() would have done)."""
    if "antenv.axon_hooks" in sys.modules:
        return
    import types

    import antenv

    mod = types.ModuleType("antenv.axon_hooks")
    state = {"hook": None}
    mod.set_axon_ntff_profile_hook = lambda h: state.__setitem__("hook", h)
    mod.get_axon_ntff_profile_hook = lambda: state["hook"]
    sys.modules["antenv.axon_hooks"] = mod
    antenv.axon_hooks = mod

    from trn_agent_boot.trn_boot import _ntff_profile_via_ctypes

    mod.set_axon_ntff_profile_hook(
        _ntff_profile_via_ctypes("/opt/axon/libaxon_pjrt.so")
    )

    # no S3 in this container — keep artifacts local
    from concourse import bass_utils

    bass_utils.upload_artifacts = lambda tmpdir: tmpdir


def _run(x, decay_logit, trace=False):
    from concourse.bass_utils import run_bass_kernel_spmd

    if trace:
        _install_profile_hook()

    x = np.ascontiguousarray(np.asarray(x, dtype=np.float32))
    assert x.shape == (B, T, D), x.shape
    lt_all = _build_weights(decay_logit)

    nc = _get_program()
    in_maps = [
        {"x": np.ascontiguousarray(x[b]), "lt_all": lt_all} for b in range(N_CORES)
    ]
    res = run_bass_kernel_spmd(
        nc,
        in_maps,
        core_ids=list(range(N_CORES)),
        trace=trace,
        trace_cores=[0] if trace else None,
    )
    y = np.stack([res.results[b]["y"] for b in range(N_CORES)], axis=0)
    return y, res


def kernel(x, decay_logit):
    y, _ = _run(x, decay_logit, trace=False)
    return y


def kernel_traced(x, decay_logit):
    """Like kernel() but returns (y, BassKernelResults) with NTFF profile."""
    return _run(x, decay_logit, trace=True)
